# revision 2
# baseline (speedup 1.0000x reference)
"""Greedy LSTM decoder (B=64, H=1024, V=32000, T<=40) on 8 Trainium2 cores.

Strategy (tensor-parallel over both H and V):
  - LSTM hidden dim sharded 8 ways: core k computes gates/h2/c2 for hidden
    units [k*128, (k+1)*128). The x-projection (embed[sym] @ w_ih.T + biases)
    is precomputed on the host as a table E'' = embed @ w_ih.T + b_ih + b_hh,
    so the per-step x-contribution is a 64-row indirect-DMA gather.
  - h2 slices are AllGathered each step (each core needs full h for both the
    recurrent matmul and the output projection).
  - Output projection sharded over vocab: core k holds w_out rows
    [k*4000, (k+1)*4000) (transposed, SBUF-resident) and computes its logit
    shard [64, 4000] per step, plus a local argmax (DVE max/max_index).
  - Local (value, index) argmax pairs are AllGathered and every core picks the
    global greedy symbol, which indexes the next gather.

All matmuls run in fp32 (PE 2-pass mode, ~1e-6 abs err) because the reference
trajectory has top-2 logit gaps down to 4e-7 and any argmax flip diverges the
whole sequence.
"""

import numpy as np

SOS = 1
B, H, V = 64, 1024, 32000
NC = 8
VS = V // NC          # 4000 vocab shard
HS = H // NC          # 128 hidden shard
GS = 4 * HS           # 512 gate slice per core
NCHUNK = 8
CW = VS // NCHUNK     # 500 logit chunk width

_cache = {}           # S -> (nc, input names) compiled program
TRACE = False         # test harness may set kernel.TRACE = True
LAST_EXEC_NS = None
LAST_RESULTS = None


def _gate_perm(k):
    """Rows of [4H, H]-shaped gate weight matrices owned by core k, reordered
    [i, f, o, g] so one sigmoid covers [0:384] and tanh covers [384:512]."""
    s = np.arange(k * HS, (k + 1) * HS)
    return np.concatenate([0 * H + s, 1 * H + s, 3 * H + s, 2 * H + s])


def _build(S):
    """Build the bass program for S LSTM steps (t = 1..S)."""
    import concourse.bass as bass
    import concourse.bacc as bacc
    import concourse.tile as tile
    import concourse.mybir as mybir
    from concourse.masks import make_identity

    F32 = mybir.dt.float32
    U32 = mybir.dt.uint32
    AF = mybir.ActivationFunctionType
    OP = mybir.AluOpType

    nc = bacc.Bacc("TRN2", target_bir_lowering=False, debug=False, num_devices=NC)

    epp = nc.dram_tensor("epp", [V, GS], F32, kind="ExternalInput").ap()
    wgT = nc.dram_tensor("wgT", [H, GS], F32, kind="ExternalInput").ap()
    woT = nc.dram_tensor("woT", [H, VS], F32, kind="ExternalInput").ap()
    bo = nc.dram_tensor("bo", [1, VS], F32, kind="ExternalInput").ap()
    h0T = nc.dram_tensor("h0T", [H, B], F32, kind="ExternalInput").ap()
    c0s = nc.dram_tensor("c0s", [B, HS], F32, kind="ExternalInput").ap()
    voff = nc.dram_tensor("voff", [B, 1], F32, kind="ExternalInput").ap()
    o_decs = nc.dram_tensor("decs", [B, S, VS], F32, kind="ExternalOutput").ap()
    o_syms = nc.dram_tensor("syms", [B, S], F32, kind="ExternalOutput").ap()

    rg = [list(range(NC))]

    with tile.TileContext(nc) as tc:
        with (
            tc.tile_pool(name="cst", bufs=1) as cst,
            tc.tile_pool(name="sb", bufs=2) as sb,
            tc.tile_pool(name="lgp", bufs=1) as lgp,
            tc.tile_pool(name="psg", bufs=2, space="PSUM") as psg,
            tc.tile_pool(name="psl", bufs=3, space="PSUM") as psl,
            tc.tile_pool(name="pst", bufs=2, space="PSUM") as pst,
            tc.tile_pool(name="dram", bufs=3, space="DRAM") as dramp,
        ):
            # ---- persistent/resident data ----
            woT_sb = cst.tile([128, 8, VS], F32)
            nc.sync.dma_start(woT_sb[:], woT.rearrange("(k p) n -> p k n", p=128))
            wgT_sb = cst.tile([128, 8, GS], F32)
            nc.sync.dma_start(wgT_sb[:], wgT.rearrange("(k p) n -> p k n", p=128))
            bo_sb = cst.tile([1, VS], F32)
            nc.sync.dma_start(bo_sb[:], bo)
            voff_sb = cst.tile([B, 1], F32)
            nc.sync.dma_start(voff_sb[:], voff)
            ident = cst.tile([128, 128], F32)
            make_identity(nc, ident[:])
            ones = cst.tile([1, B], F32)
            nc.vector.memset(ones[:], 1.0)
            big = cst.tile([B, 8], F32)
            nc.vector.memset(big[:], 1.0e9)
            syms_acc = cst.tile([B, S], F32)

            hT_sb = sb.tile([128, 8, B], F32, tag="hT")
            nc.sync.dma_start(hT_sb[:], h0T.rearrange("(k p) b -> p k b", p=128))
            c_cur = sb.tile([B, HS], F32, tag="c")
            nc.sync.dma_start(c_cur[:], c0s)
            sym_u32 = sb.tile([B, 1], U32, tag="symu")
            nc.vector.memset(sym_u32[:], SOS)

            for s in range(S):
                # ---- gates-x gather: gx = E''[sym]  [B, GS] ----
                gx = sb.tile([B, GS], F32, tag="gx")
                nc.gpsimd.indirect_dma_start(
                    out=gx[:], out_offset=None, in_=epp,
                    in_offset=bass.IndirectOffsetOnAxis(ap=sym_u32[:, :1], axis=0))

                # ---- gates-h matmul ----
                g_ps = psg.tile([B, GS], F32, tag="g")
                for k in range(8):
                    nc.tensor.matmul(g_ps[:], lhsT=hT_sb[:, k, :],
                                     rhs=wgT_sb[:, k, :],
                                     start=(k == 0), stop=(k == 7))
                g_sb = sb.tile([B, GS], F32, tag="gsb")
                nc.vector.tensor_tensor(out=g_sb[:], in0=g_ps[:], in1=gx[:], op=OP.add)

                # ---- LSTM elementwise (gate order i,f,o,g) ----
                sig = sb.tile([B, 3 * HS], F32, tag="sig")
                nc.scalar.activation(sig[:], g_sb[:, 0:3 * HS], AF.Sigmoid)
                tg = sb.tile([B, HS], F32, tag="tg")
                nc.scalar.activation(tg[:], g_sb[:, 3 * HS:4 * HS], AF.Tanh)
                t1 = sb.tile([B, HS], F32, tag="t1")
                nc.vector.tensor_tensor(out=t1[:], in0=sig[:, 0:HS], in1=tg[:], op=OP.mult)
                t2 = sb.tile([B, HS], F32, tag="t2")
                nc.vector.tensor_tensor(out=t2[:], in0=sig[:, HS:2 * HS], in1=c_cur[:], op=OP.mult)
                c_new = sb.tile([B, HS], F32, tag="c")
                nc.vector.tensor_tensor(out=c_new[:], in0=t1[:], in1=t2[:], op=OP.add)
                c_cur = c_new
                tc2 = sb.tile([B, HS], F32, tag="tc2")
                nc.scalar.activation(tc2[:], c_new[:], AF.Tanh)
                h2 = sb.tile([B, HS], F32, tag="h2")
                nc.vector.tensor_tensor(out=h2[:], in0=sig[:, 2 * HS:3 * HS], in1=tc2[:], op=OP.mult)

                # ---- h2 -> h2T, AllGather full h ----
                trp = pst.tile([HS, B], F32, tag="tr")
                nc.tensor.transpose(out=trp[:], in_=h2[:], identity=ident[:B, :B])
                h2T = sb.tile([HS, B], F32, tag="h2T")
                nc.vector.tensor_copy(out=h2T[:], in_=trp[:])
                agi = dramp.tile([HS, B], F32, tag="agi")
                nc.sync.dma_start(agi[:], h2T[:])
                ago = dramp.tile([H, B], F32, tag="ago", addr_space="Shared")
                nc.gpsimd.collective_compute(
                    "AllGather", OP.bypass, replica_groups=rg,
                    ins=[agi[:].opt()], outs=[ago[:].opt()])
                hT_sb = sb.tile([128, 8, B], F32, tag="hT")
                nc.sync.dma_start(hT_sb[:], ago[:].rearrange("(k p) b -> p k b", p=128))

                # ---- output projection (vocab shard) ----
                logits_sb = lgp.tile([B, VS], F32, tag="lg")
                for nci in range(NCHUNK):
                    cs = slice(nci * CW, (nci + 1) * CW)
                    lp = psl.tile([B, CW], F32, tag="lp")
                    nc.tensor.matmul(lp[:], lhsT=ones[:, :], rhs=bo_sb[0:1, cs],
                                     start=True, stop=False)
                    for k in range(8):
                        nc.tensor.matmul(lp[:], lhsT=hT_sb[:, k, :],
                                         rhs=woT_sb[:, k, cs],
                                         start=False, stop=(k == 7))
                    nc.scalar.activation(logits_sb[:, cs], lp[:], AF.Copy)
                nc.sync.dma_start(o_decs[:, s, :], logits_sb[:])

                # ---- local argmax over the vocab shard ----
                mx8 = sb.tile([B, 8], F32, tag="mx8")
                nc.vector.max(out=mx8[:], in_=logits_sb[:])
                ix8 = sb.tile([B, 8], U32, tag="ix8")
                nc.vector.max_index(out=ix8[:], in_max=mx8[:], in_values=logits_sb[:])
                lidx = sb.tile([B, 1], F32, tag="lidx")
                nc.vector.tensor_copy(out=lidx[:], in_=ix8[:, 0:1])
                gidx = sb.tile([B, 1], F32, tag="gidx")
                nc.vector.tensor_scalar(out=gidx[:], in0=lidx[:], scalar1=voff_sb[:, 0:1],
                                        scalar2=None, op0=OP.add)

                # ---- exchange (value, index) pairs, pick global argmax ----
                pair = sb.tile([B, 2], F32, tag="pair")
                nc.vector.tensor_copy(out=pair[:, 0:1], in_=mx8[:, 0:1])
                nc.vector.tensor_copy(out=pair[:, 1:2], in_=gidx[:])
                prT = pst.tile([2, B], F32, tag="tr")
                nc.tensor.transpose(out=prT[:], in_=pair[:], identity=ident[:B, :B])
                pairT = sb.tile([2, B], F32, tag="pairT")
                nc.vector.tensor_copy(out=pairT[:], in_=prT[:])
                agi2 = dramp.tile([2, B], F32, tag="agi2")
                nc.sync.dma_start(agi2[:], pairT[:])
                ago2 = dramp.tile([2 * NC, B], F32, tag="ago2", addr_space="Shared")
                nc.gpsimd.collective_compute(
                    "AllGather", OP.bypass, replica_groups=rg,
                    ins=[agi2[:].opt()], outs=[ago2[:].opt()])
                cand = sb.tile([2 * NC, B], F32, tag="cand")
                nc.sync.dma_start(cand[:], ago2[:])
                cndT = pst.tile([B, 2 * NC], F32, tag="tr")
                nc.tensor.transpose(out=cndT[:], in_=cand[:], identity=ident[:2 * NC, :2 * NC])
                candT = sb.tile([B, 2 * NC], F32, tag="candT")
                nc.vector.tensor_copy(out=candT[:], in_=cndT[:])

                cv = candT[:].rearrange("b (c two) -> b c two", two=2)
                vals8 = cv[:, :, 0]
                idxs8 = cv[:, :, 1]
                gm8 = sb.tile([B, 8], F32, tag="gm8")
                nc.vector.max(out=gm8[:], in_=vals8)
                eq = sb.tile([B, 8], U32, tag="eq")
                nc.vector.tensor_scalar(out=eq[:], in0=vals8, scalar1=gm8[:, 0:1],
                                        scalar2=None, op0=OP.is_equal)
                sel = sb.tile([B, 8], F32, tag="sel")
                nc.vector.select(out=sel[:], mask=eq[:], on_true=idxs8, on_false=big[:])
                neg = sb.tile([B, 8], F32, tag="neg")
                nc.vector.tensor_scalar(out=neg[:], in0=sel[:], scalar1=-1.0,
                                        scalar2=None, op0=OP.mult)
                nm8 = sb.tile([B, 8], F32, tag="nm8")
                nc.vector.max(out=nm8[:], in_=neg[:])
                symf = sb.tile([B, 1], F32, tag="symf")
                nc.vector.tensor_scalar(out=symf[:], in0=nm8[:, 0:1], scalar1=-1.0,
                                        scalar2=None, op0=OP.mult)
                nc.vector.tensor_copy(out=syms_acc[:, s:s + 1], in_=symf[:])
                if s < S - 1:
                    sym_u32 = sb.tile([B, 1], U32, tag="symu")
                    nc.vector.tensor_copy(out=sym_u32[:], in_=symf[:])

            nc.sync.dma_start(o_syms, syms_acc[:])

    nc.compile()
    return nc


def _get_program(S):
    if S not in _cache:
        _cache[S] = _build(S)
    return _cache[S]


def _install_ntff_hook():
    import sys, types
    if "antenv.axon_hooks" in sys.modules:
        return
    try:
        from trn_agent_boot.trn_boot import _ntff_profile_via_ctypes
        hook = _ntff_profile_via_ctypes("/opt/axon/libaxon_pjrt.so")
    except Exception:
        return
    import antenv
    mod = types.ModuleType("antenv.axon_hooks")
    mod._hook = hook
    mod.set_axon_ntff_profile_hook = lambda h: setattr(mod, "_hook", h)
    mod.get_axon_ntff_profile_hook = lambda: mod._hook
    sys.modules["antenv.axon_hooks"] = mod
    antenv.axon_hooks = mod


def kernel(output, h0, c0, target_outputs, target_lengths,
           embed, w_ih, w_hh, b_ih, b_hh, w_out, b_out):
    global LAST_EXEC_NS, LAST_RESULTS
    import concourse.bass_utils as bass_utils

    embed = np.asarray(embed, dtype=np.float32)
    w_ih = np.asarray(w_ih, dtype=np.float32)
    w_hh = np.asarray(w_hh, dtype=np.float32)
    b_ih = np.asarray(b_ih, dtype=np.float32)
    b_hh = np.asarray(b_hh, dtype=np.float32)
    w_out = np.asarray(w_out, dtype=np.float32)
    b_out = np.asarray(b_out, dtype=np.float32)
    h0 = np.asarray(h0, dtype=np.float32)
    c0 = np.asarray(c0, dtype=np.float32)
    tl = np.asarray(target_lengths)
    sym_dtype = np.asarray(target_outputs).dtype

    T = int(tl.max()) if tl.size else 0
    S = max(T - 1, 0)

    # host precompute
    dec0 = (embed[SOS] @ w_out.T + b_out).astype(np.float32)      # [V]
    if T == 0:
        decs = np.zeros((B, 0, V), np.float32)
        syms = np.zeros((B, 0), sym_dtype)
        return decs, syms

    decs = np.empty((B, T, V), np.float32)
    decs[:, 0, :] = dec0[None, :]
    syms = np.empty((B, T), np.int64)
    syms[:, 0] = SOS
    if S == 0:
        return decs, syms.astype(sym_dtype)

    Epp = (embed @ w_ih.T + (b_ih + b_hh)[None, :]).astype(np.float32)  # [V, 4H]
    h0T = np.ascontiguousarray(h0[0].T)                                  # [H, B]

    in_maps = []
    for k in range(NC):
        perm = _gate_perm(k)
        in_maps.append({
            "epp": np.ascontiguousarray(Epp[:, perm]),
            "wgT": np.ascontiguousarray(w_hh[perm, :].T),
            "woT": np.ascontiguousarray(w_out[k * VS:(k + 1) * VS, :].T),
            "bo": np.ascontiguousarray(b_out[None, k * VS:(k + 1) * VS]),
            "h0T": h0T,
            "c0s": np.ascontiguousarray(c0[0][:, k * HS:(k + 1) * HS]),
            "voff": np.full((B, 1), k * VS, np.float32),
        })

    nc = _get_program(S)
    if TRACE:
        _install_ntff_hook()
    res = bass_utils.run_bass_kernel_spmd(
        nc, in_maps, core_ids=list(range(NC)), trace=TRACE)
    LAST_EXEC_NS = res.exec_time_ns
    LAST_RESULTS = res

    for k in range(NC):
        decs[:, 1:, k * VS:(k + 1) * VS] = res.results[k]["decs"]
    syms[:, 1:] = np.rint(res.results[0]["syms"]).astype(np.int64)
    return decs, syms.astype(sym_dtype)


# revision 5
# speedup vs baseline: 1.0993x; 1.0993x over previous
"""Greedy LSTM decoder (B=64, H=1024, V=32000, T<=40) on 8 Trainium2 cores.

Strategy (tensor-parallel over both H and V):
  - LSTM hidden dim sharded 8 ways: core k computes gates/h2/c2 for hidden
    units [k*128, (k+1)*128). The x-projection (embed[sym] @ w_ih.T + biases)
    is precomputed on the host as a table E'' = embed @ w_ih.T + b_ih + b_hh,
    so the per-step x-contribution is a 64-row indirect-DMA gather.
  - h2 slices are AllGathered each step (each core needs full h for both the
    recurrent matmul and the output projection).
  - Output projection sharded over vocab: core k holds w_out rows
    [k*4000, (k+1)*4000) (transposed, SBUF-resident) and computes its logit
    shard [64, 4000] per step, plus a local argmax (DVE max/max_index,
    chunked so it hides under the projection matmuls).
  - Local (value, index) argmax pairs are AllGathered and every core picks the
    global greedy symbol, which indexes the next gather.

All matmuls run in fp32 (PE 2-pass mode, ~1e-6 abs err) because the reference
trajectory has top-2 logit gaps down to 4e-7 and any argmax flip diverges the
whole sequence. Transposes run on DVE (32x32 stream transpose) to keep the
in-order PE stream free of argmax/collective dependencies.
"""

import numpy as np

SOS = 1
B, H, V = 64, 1024, 32000
NC = 8
VS = V // NC          # 4000 vocab shard
HS = H // NC          # 128 hidden shard
GS = 4 * HS           # 512 gate slice per core
NCHUNK = 8
CW = VS // NCHUNK     # 500 logit chunk width

_cache = {}           # S -> compiled program
TRACE = False         # test harness may set kernel.TRACE = True
LAST_EXEC_NS = None
LAST_RESULTS = None


def _gate_perm(k):
    """Rows of [4H, H]-shaped gate weight matrices owned by core k, reordered
    [i, f, o, g] so one sigmoid covers [0:384] and tanh covers [384:512]."""
    s = np.arange(k * HS, (k + 1) * HS)
    return np.concatenate([0 * H + s, 1 * H + s, 3 * H + s, 2 * H + s])


def _dve_transpose(nc, out, in_, p, f):
    """Global transpose in_[p, f] -> out[f, p] via DVE 32x32 block transposes.
    p, f must be multiples of 32."""
    for i in range(p // 32):
        for j in range(f // 32):
            nc.vector.transpose(out=out[j * 32:(j + 1) * 32, i * 32:(i + 1) * 32],
                                in_=in_[i * 32:(i + 1) * 32, j * 32:(j + 1) * 32])


def _build(S):
    """Build the bass program for S LSTM steps (t = 1..S)."""
    import concourse.bass as bass
    import concourse.bacc as bacc
    import concourse.tile as tile
    import concourse.mybir as mybir

    F32 = mybir.dt.float32
    U32 = mybir.dt.uint32
    AF = mybir.ActivationFunctionType
    OP = mybir.AluOpType

    nc = bacc.Bacc("TRN2", target_bir_lowering=False, debug=False, num_devices=NC)

    epp = nc.dram_tensor("epp", [V, GS], F32, kind="ExternalInput").ap()
    wgT = nc.dram_tensor("wgT", [H, GS], F32, kind="ExternalInput").ap()
    woT = nc.dram_tensor("woT", [H, VS], F32, kind="ExternalInput").ap()
    bo = nc.dram_tensor("bo", [1, VS], F32, kind="ExternalInput").ap()
    h0T = nc.dram_tensor("h0T", [H, B], F32, kind="ExternalInput").ap()
    c0s = nc.dram_tensor("c0s", [B, HS], F32, kind="ExternalInput").ap()
    voff = nc.dram_tensor("voff", [B, 1], F32, kind="ExternalInput").ap()
    o_decs = nc.dram_tensor("decs", [B, S, VS], F32, kind="ExternalOutput").ap()
    o_syms = nc.dram_tensor("syms", [B, S], F32, kind="ExternalOutput").ap()

    rg = [list(range(NC))]

    with tile.TileContext(nc) as tc:
        with (
            tc.tile_pool(name="cst", bufs=1) as cst,
            tc.tile_pool(name="sb", bufs=2) as sb,
            tc.tile_pool(name="lgp", bufs=1) as lgp,
            tc.tile_pool(name="psg", bufs=2, space="PSUM") as psg,
            tc.tile_pool(name="psl", bufs=1, space="PSUM") as psl,
            tc.tile_pool(name="dram", bufs=3, space="DRAM") as dramp,
        ):
            # ---- persistent/resident data ----
            woT_sb = cst.tile([128, 8, VS], F32)
            nc.sync.dma_start(woT_sb[:], woT.rearrange("(k p) n -> p k n", p=128))
            wgT_sb = cst.tile([128, 8, GS], F32)
            nc.sync.dma_start(wgT_sb[:], wgT.rearrange("(k p) n -> p k n", p=128))
            bo64_sb = cst.tile([B, VS], F32)
            nc.sync.dma_start(bo64_sb[:], bo.to_broadcast([B, VS]))
            voff_sb = cst.tile([B, 1], F32)
            nc.sync.dma_start(voff_sb[:], voff)
            big = cst.tile([B, 8], F32)
            nc.vector.memset(big[:], 1.0e9)
            choff = cst.tile([B, 8], F32)
            for c in range(NCHUNK):
                nc.vector.memset(choff[:, c:c + 1], float(c * CW))
            syms_acc = cst.tile([B, S], F32)

            hT_sb = sb.tile([128, 8, B], F32, tag="hT")
            nc.sync.dma_start(hT_sb[:], h0T.rearrange("(k p) b -> p k b", p=128))
            c_cur = sb.tile([B, HS], F32, tag="c")
            nc.sync.dma_start(c_cur[:], c0s)
            sym_u32 = sb.tile([B, 1], U32, tag="symu")
            nc.vector.memset(sym_u32[:], SOS)

            for s in range(S):
                # ---- gates-x gather: gx = E''[sym]  [B, GS] ----
                gx = sb.tile([B, GS], F32, tag="gx")
                nc.gpsimd.indirect_dma_start(
                    out=gx[:], out_offset=None, in_=epp,
                    in_offset=bass.IndirectOffsetOnAxis(ap=sym_u32[:, :1], axis=0))

                # ---- gates-h matmul ----
                g_ps = psg.tile([B, GS], F32, tag="g")
                for k in range(8):
                    nc.tensor.matmul(g_ps[:], lhsT=hT_sb[:, k, :],
                                     rhs=wgT_sb[:, k, :],
                                     start=(k == 0), stop=(k == 7))
                g_sb = sb.tile([B, GS], F32, tag="gsb")
                nc.vector.tensor_tensor(out=g_sb[:], in0=g_ps[:], in1=gx[:], op=OP.add)

                # ---- LSTM elementwise (gate order i,f,o,g) ----
                sig = sb.tile([B, 3 * HS], F32, tag="sig")
                nc.scalar.activation(sig[:], g_sb[:, 0:3 * HS], AF.Sigmoid)
                tg = sb.tile([B, HS], F32, tag="tg")
                nc.scalar.activation(tg[:], g_sb[:, 3 * HS:4 * HS], AF.Tanh)
                t1 = sb.tile([B, HS], F32, tag="t1")
                nc.vector.tensor_tensor(out=t1[:], in0=sig[:, 0:HS], in1=tg[:], op=OP.mult)
                t2 = sb.tile([B, HS], F32, tag="t2")
                nc.vector.tensor_tensor(out=t2[:], in0=sig[:, HS:2 * HS], in1=c_cur[:], op=OP.mult)
                c_new = sb.tile([B, HS], F32, tag="c")
                nc.vector.tensor_tensor(out=c_new[:], in0=t1[:], in1=t2[:], op=OP.add)
                c_cur = c_new
                tc2 = sb.tile([B, HS], F32, tag="tc2")
                nc.scalar.activation(tc2[:], c_new[:], AF.Tanh)
                h2 = sb.tile([B, HS], F32, tag="h2")
                nc.vector.tensor_tensor(out=h2[:], in0=sig[:, 2 * HS:3 * HS], in1=tc2[:], op=OP.mult)

                # ---- h2 -> h2T (DVE), AllGather full h ----
                h2T = sb.tile([HS, B], F32, tag="h2T")
                _dve_transpose(nc, h2T[:], h2[:], B, HS)
                agi = dramp.tile([HS, B], F32, tag="agi")
                nc.sync.dma_start(agi[:], h2T[:])
                ago = dramp.tile([H, B], F32, tag="ago", addr_space="Shared")
                nc.gpsimd.collective_compute(
                    "AllGather", OP.bypass, replica_groups=rg,
                    ins=[agi[:].opt()], outs=[ago[:].opt()])
                hT_sb = sb.tile([128, 8, B], F32, tag="hT")
                nc.sync.dma_start(hT_sb[:], ago[:].rearrange("(k p) b -> p k b", p=128))

                # ---- output projection (vocab shard), k-outer over 4-chunk halves ----
                logits_sb = lgp.tile([B, VS], F32, tag="lg")
                mx8_all = sb.tile([B, NCHUNK, 8], F32, tag="mx8a")
                ix8_all = sb.tile([B, NCHUNK, 8], U32, tag="ix8a")
                for half in range(2):
                    lps = [psl.tile([B, CW], F32, tag=f"lp{i}", name=f"lp{i}")
                           for i in range(4)]
                    for k in range(8):
                        for i in range(4):
                            nci = half * 4 + i
                            cs = slice(nci * CW, (nci + 1) * CW)
                            nc.tensor.matmul(lps[i][:], lhsT=hT_sb[:, k, :],
                                             rhs=woT_sb[:, k, cs],
                                             start=(k == 0), stop=(k == 7))
                    for i in range(4):
                        nci = half * 4 + i
                        cs = slice(nci * CW, (nci + 1) * CW)
                        nc.vector.tensor_tensor(out=logits_sb[:, cs], in0=lps[i][:],
                                                in1=bo64_sb[:, cs], op=OP.add)
                        nc.vector.max(out=mx8_all[:, nci, :], in_=logits_sb[:, cs])
                        nc.vector.max_index(out=ix8_all[:, nci, :],
                                            in_max=mx8_all[:, nci, :],
                                            in_values=logits_sb[:, cs])
                nc.sync.dma_start(o_decs[:, s, :], logits_sb[:])

                # ---- merge chunk argmaxes (local shard winner) ----
                vals = mx8_all[:, :, 0]                       # [B, 8] stride-8 AP
                ix8f = sb.tile([B, 8], F32, tag="ix8f")
                nc.vector.tensor_copy(out=ix8f[:], in_=ix8_all[:, :, 0])
                idxg = sb.tile([B, 8], F32, tag="idxg")
                nc.vector.tensor_tensor(out=idxg[:], in0=ix8f[:], in1=choff[:], op=OP.add)
                gm8 = sb.tile([B, 8], F32, tag="gm8")
                nc.vector.max(out=gm8[:], in_=vals)
                eq = sb.tile([B, 8], U32, tag="eq")
                nc.vector.tensor_scalar(out=eq[:], in0=vals, scalar1=gm8[:, 0:1],
                                        scalar2=None, op0=OP.is_equal)
                sel = sb.tile([B, 8], F32, tag="sel")
                nc.vector.select(out=sel[:], mask=eq[:], on_true=idxg[:], on_false=big[:])
                neg = sb.tile([B, 8], F32, tag="neg")
                nc.vector.tensor_scalar(out=neg[:], in0=sel[:], scalar1=-1.0,
                                        scalar2=None, op0=OP.mult)
                nm8 = sb.tile([B, 8], F32, tag="nm8")
                nc.vector.max(out=nm8[:], in_=neg[:])
                negl = sb.tile([B, 1], F32, tag="negl")
                nc.vector.tensor_scalar(out=negl[:], in0=nm8[:, 0:1], scalar1=-1.0,
                                        scalar2=None, op0=OP.mult)
                gidx = sb.tile([B, 1], F32, tag="gidx")
                nc.vector.tensor_scalar(out=gidx[:], in0=negl[:], scalar1=voff_sb[:, 0:1],
                                        scalar2=None, op0=OP.add)

                # ---- exchange (value, index) pairs, pick global argmax ----
                pair = sb.tile([B, 32], F32, tag="pair")
                nc.vector.tensor_copy(out=pair[:, 0:1], in_=gm8[:, 0:1])
                nc.vector.tensor_copy(out=pair[:, 1:2], in_=gidx[:])
                pairT = sb.tile([32, B], F32, tag="pairT")
                nc.vector.transpose(out=pairT[:, 0:32], in_=pair[0:32, :])
                nc.vector.transpose(out=pairT[:, 32:64], in_=pair[32:64, :])
                agi2 = dramp.tile([2, B], F32, tag="agi2")
                nc.sync.dma_start(agi2[:], pairT[0:2, :])
                ago2 = dramp.tile([2 * NC, B], F32, tag="ago2", addr_space="Shared")
                nc.gpsimd.collective_compute(
                    "AllGather", OP.bypass, replica_groups=rg,
                    ins=[agi2[:].opt()], outs=[ago2[:].opt()])
                cand32 = sb.tile([32, B], F32, tag="cand")
                nc.sync.dma_start(cand32[0:2 * NC, :], ago2[:])
                candT = sb.tile([B, 32], F32, tag="candT")
                nc.vector.transpose(out=candT[0:32, :], in_=cand32[:, 0:32])
                nc.vector.transpose(out=candT[32:64, :], in_=cand32[:, 32:64])

                cv = candT[:, 0:2 * NC].rearrange("b (c two) -> b c two", two=2)
                vals8 = cv[:, :, 0]
                idxs8 = cv[:, :, 1]
                gm8b = sb.tile([B, 8], F32, tag="gm8b")
                nc.vector.max(out=gm8b[:], in_=vals8)
                eqb = sb.tile([B, 8], U32, tag="eqb")
                nc.vector.tensor_scalar(out=eqb[:], in0=vals8, scalar1=gm8b[:, 0:1],
                                        scalar2=None, op0=OP.is_equal)
                selb = sb.tile([B, 8], F32, tag="selb")
                nc.vector.select(out=selb[:], mask=eqb[:], on_true=idxs8, on_false=big[:])
                negb = sb.tile([B, 8], F32, tag="negb")
                nc.vector.tensor_scalar(out=negb[:], in0=selb[:], scalar1=-1.0,
                                        scalar2=None, op0=OP.mult)
                nmb = sb.tile([B, 8], F32, tag="nmb")
                nc.vector.max(out=nmb[:], in_=negb[:])
                symf = sb.tile([B, 1], F32, tag="symf")
                nc.vector.tensor_scalar(out=symf[:], in0=nmb[:, 0:1], scalar1=-1.0,
                                        scalar2=None, op0=OP.mult)
                nc.vector.tensor_copy(out=syms_acc[:, s:s + 1], in_=symf[:])
                if s < S - 1:
                    sym_u32 = sb.tile([B, 1], U32, tag="symu")
                    nc.vector.tensor_copy(out=sym_u32[:], in_=symf[:])

            nc.sync.dma_start(o_syms, syms_acc[:])

    nc.compile()
    return nc


def _get_program(S):
    if S not in _cache:
        _cache[S] = _build(S)
    return _cache[S]


def _install_ntff_hook():
    import sys, types
    if "antenv.axon_hooks" in sys.modules:
        return
    try:
        from trn_agent_boot.trn_boot import _ntff_profile_via_ctypes
        hook = _ntff_profile_via_ctypes("/opt/axon/libaxon_pjrt.so")
    except Exception:
        return
    import antenv
    mod = types.ModuleType("antenv.axon_hooks")
    mod._hook = hook
    mod.set_axon_ntff_profile_hook = lambda h: setattr(mod, "_hook", h)
    mod.get_axon_ntff_profile_hook = lambda: mod._hook
    sys.modules["antenv.axon_hooks"] = mod
    antenv.axon_hooks = mod


def kernel(output, h0, c0, target_outputs, target_lengths,
           embed, w_ih, w_hh, b_ih, b_hh, w_out, b_out):
    global LAST_EXEC_NS, LAST_RESULTS
    import concourse.bass_utils as bass_utils

    embed = np.asarray(embed, dtype=np.float32)
    w_ih = np.asarray(w_ih, dtype=np.float32)
    w_hh = np.asarray(w_hh, dtype=np.float32)
    b_ih = np.asarray(b_ih, dtype=np.float32)
    b_hh = np.asarray(b_hh, dtype=np.float32)
    w_out = np.asarray(w_out, dtype=np.float32)
    b_out = np.asarray(b_out, dtype=np.float32)
    h0 = np.asarray(h0, dtype=np.float32)
    c0 = np.asarray(c0, dtype=np.float32)
    tl = np.asarray(target_lengths)
    sym_dtype = np.asarray(target_outputs).dtype

    T = int(tl.max()) if tl.size else 0
    S = max(T - 1, 0)

    dec0 = (embed[SOS] @ w_out.T + b_out).astype(np.float32)      # [V]
    if T == 0:
        return np.zeros((B, 0, V), np.float32), np.zeros((B, 0), sym_dtype)

    decs = np.empty((B, T, V), np.float32)
    decs[:, 0, :] = dec0[None, :]
    syms = np.empty((B, T), np.int64)
    syms[:, 0] = SOS
    if S == 0:
        return decs, syms.astype(sym_dtype)

    Epp = (embed @ w_ih.T + (b_ih + b_hh)[None, :]).astype(np.float32)  # [V, 4H]
    h0T = np.ascontiguousarray(h0[0].T)                                  # [H, B]

    in_maps = []
    for k in range(NC):
        perm = _gate_perm(k)
        in_maps.append({
            "epp": np.ascontiguousarray(Epp[:, perm]),
            "wgT": np.ascontiguousarray(w_hh[perm, :].T),
            "woT": np.ascontiguousarray(w_out[k * VS:(k + 1) * VS, :].T),
            "bo": np.ascontiguousarray(b_out[None, k * VS:(k + 1) * VS]),
            "h0T": h0T,
            "c0s": np.ascontiguousarray(c0[0][:, k * HS:(k + 1) * HS]),
            "voff": np.full((B, 1), k * VS, np.float32),
        })

    nc = _get_program(S)
    if TRACE:
        _install_ntff_hook()
    res = bass_utils.run_bass_kernel_spmd(
        nc, in_maps, core_ids=list(range(NC)), trace=TRACE)
    LAST_EXEC_NS = res.exec_time_ns
    LAST_RESULTS = res

    for k in range(NC):
        decs[:, 1:, k * VS:(k + 1) * VS] = res.results[k]["decs"]
    syms[:, 1:] = np.rint(res.results[0]["syms"]).astype(np.int64)
    return decs, syms.astype(sym_dtype)


# revision 10
# speedup vs baseline: 1.2375x; 1.1257x over previous
"""Greedy LSTM decoder (B=64, H=1024, V=32000, T<=40) on 8 Trainium2 cores.

Strategy (tensor-parallel over both H and V):
  - LSTM hidden dim sharded 8 ways: core k computes gates/h2/c2 for hidden
    units [k*128, (k+1)*128). The x-projection (embed[sym] @ w_ih.T + biases)
    is precomputed on the host as a table E'' = embed @ w_ih.T + b_ih + b_hh,
    so the per-step x-contribution is a 64-row indirect-DMA gather.
  - h2 slices are AllGathered each step (each core needs full h for both the
    recurrent matmul and the output projection).
  - Output projection sharded over vocab: core k holds w_out rows
    [k*4000, (k+1)*4000) (transposed, SBUF-resident) and computes its logit
    shard [64, 4000] per step, plus a local argmax (DVE max/max_index,
    chunked so it hides under the projection matmuls).
  - Local (value, index) argmax pairs are AllGathered and every core picks the
    global greedy symbol, which indexes the next gather.

All matmuls run in fp32 (PE 2-pass mode, ~1e-6 abs err) because the reference
trajectory has top-2 logit gaps down to 4e-7 and any argmax flip diverges the
whole sequence. Transposes run on DVE (32x32 stream transpose) to keep the
in-order PE stream free of argmax/collective dependencies.
"""

import numpy as np

SOS = 1
B, H, V = 64, 1024, 32000
NC = 8
VS = V // NC          # 4000 vocab shard
HS = H // NC          # 128 hidden shard
GS = 4 * HS           # 512 gate slice per core
NCHUNK = 8
CW = VS // NCHUNK     # 500 logit chunk width

_cache = {}           # S -> compiled program
TRACE = False         # test harness may set kernel.TRACE = True
LAST_EXEC_NS = None
LAST_RESULTS = None


def _gate_perm(k):
    """Rows of [4H, H]-shaped gate weight matrices owned by core k, reordered
    [i, f, o, g] so one sigmoid covers [0:384] and tanh covers [384:512]."""
    s = np.arange(k * HS, (k + 1) * HS)
    return np.concatenate([0 * H + s, 1 * H + s, 3 * H + s, 2 * H + s])


def _dve_transpose(nc, out, in_, p, f):
    """Global transpose in_[p, f] -> out[f, p] via DVE 32x32 block transposes.
    p, f must be multiples of 32."""
    for i in range(p // 32):
        for j in range(f // 32):
            nc.vector.transpose(out=out[j * 32:(j + 1) * 32, i * 32:(i + 1) * 32],
                                in_=in_[i * 32:(i + 1) * 32, j * 32:(j + 1) * 32])


def _build(S):
    """Build the bass program for S LSTM steps (t = 1..S)."""
    import concourse.bass as bass
    import concourse.bacc as bacc
    import concourse.tile as tile
    import concourse.mybir as mybir

    F32 = mybir.dt.float32
    U32 = mybir.dt.uint32
    AF = mybir.ActivationFunctionType
    OP = mybir.AluOpType

    nc = bacc.Bacc("TRN2", target_bir_lowering=False, debug=False, num_devices=NC)

    BF16 = mybir.dt.bfloat16
    epp = nc.dram_tensor("epp", [V, GS], F32, kind="ExternalInput").ap()
    wgT = nc.dram_tensor("wgT", [H, GS], F32, kind="ExternalInput").ap()
    whi = nc.dram_tensor("whi", [H, VS], BF16, kind="ExternalInput").ap()
    wlo = nc.dram_tensor("wlo", [H, VS], BF16, kind="ExternalInput").ap()
    bo3 = nc.dram_tensor("bo3", [3, VS], BF16, kind="ExternalInput").ap()
    h0T = nc.dram_tensor("h0T", [H, B], F32, kind="ExternalInput").ap()
    c0s = nc.dram_tensor("c0s", [B, HS], F32, kind="ExternalInput").ap()
    voff = nc.dram_tensor("voff", [B, 1], F32, kind="ExternalInput").ap()
    o_decs = nc.dram_tensor("decs", [B, S, VS], F32, kind="ExternalOutput").ap()
    o_syms = nc.dram_tensor("syms", [B, S], F32, kind="ExternalOutput").ap()

    rg = [list(range(NC))]

    with tile.TileContext(nc) as tc:
        with (
            tc.tile_pool(name="cst", bufs=1) as cst,
            tc.tile_pool(name="sb", bufs=2) as sb,
            tc.tile_pool(name="lgp", bufs=1) as lgp,
            tc.tile_pool(name="psg", bufs=2, space="PSUM") as psg,
            tc.tile_pool(name="psl", bufs=2, space="PSUM") as psl,
            tc.tile_pool(name="dram", bufs=3, space="DRAM") as dramp,
        ):
            # ---- persistent/resident data ----
            whi_sb = cst.tile([128, 8, VS], BF16)
            nc.sync.dma_start(whi_sb[:], whi.rearrange("(k p) n -> p k n", p=128))
            wlo_sb = cst.tile([128, 8, VS], BF16)
            nc.sync.dma_start(wlo_sb[:], wlo.rearrange("(k p) n -> p k n", p=128))
            wgT_sb = cst.tile([128, 8, GS], F32)
            nc.sync.dma_start(wgT_sb[:], wgT.rearrange("(k p) n -> p k n", p=128))
            bo3_sb = cst.tile([3, VS], BF16)
            nc.sync.dma_start(bo3_sb[:], bo3)
            ones3 = cst.tile([3, B], BF16)
            nc.vector.memset(ones3[:], 1.0)
            voff_sb = cst.tile([B, 1], F32)
            nc.sync.dma_start(voff_sb[:], voff)
            big = cst.tile([B, 8], F32)
            nc.vector.memset(big[:], 1.0e9)
            choff = cst.tile([B, 8], F32)
            for c in range(NCHUNK):
                nc.vector.memset(choff[:, c:c + 1], float(c * CW))
            syms_acc = cst.tile([B, S], F32)

            hT_sb = sb.tile([128, 8, B], F32, tag="hT")
            nc.sync.dma_start(hT_sb[:], h0T.rearrange("(k p) b -> p k b", p=128))
            c_cur = sb.tile([B, HS], F32, tag="c")
            nc.sync.dma_start(c_cur[:], c0s)
            sym_u32 = sb.tile([B, 1], U32, tag="symu")
            nc.vector.memset(sym_u32[:], SOS)

            for s in range(S):
                # ---- gates-x gather: gx = E''[sym]  [B, GS] ----
                gx = sb.tile([B, GS], F32, tag="gx")
                nc.gpsimd.indirect_dma_start(
                    out=gx[:], out_offset=None, in_=epp,
                    in_offset=bass.IndirectOffsetOnAxis(ap=sym_u32[:, :1], axis=0))

                # ---- gates-h matmul ----
                g_ps = psg.tile([B, GS], F32, tag="g")
                for k in range(8):
                    nc.tensor.matmul(g_ps[:], lhsT=hT_sb[:, k, :],
                                     rhs=wgT_sb[:, k, :],
                                     start=(k == 0), stop=(k == 7))
                g_sb = sb.tile([B, GS], F32, tag="gsb")
                nc.vector.tensor_tensor(out=g_sb[:], in0=g_ps[:], in1=gx[:], op=OP.add)

                # ---- LSTM elementwise (gate order i,f,o,g) ----
                sig = sb.tile([B, 3 * HS], F32, tag="sig")
                nc.scalar.activation(sig[:], g_sb[:, 0:3 * HS], AF.Sigmoid)
                tg = sb.tile([B, HS], F32, tag="tg")
                nc.scalar.activation(tg[:], g_sb[:, 3 * HS:4 * HS], AF.Tanh)
                t1 = sb.tile([B, HS], F32, tag="t1")
                nc.vector.tensor_tensor(out=t1[:], in0=sig[:, 0:HS], in1=tg[:], op=OP.mult)
                t2 = sb.tile([B, HS], F32, tag="t2")
                nc.vector.tensor_tensor(out=t2[:], in0=sig[:, HS:2 * HS], in1=c_cur[:], op=OP.mult)
                c_new = sb.tile([B, HS], F32, tag="c")
                nc.vector.tensor_tensor(out=c_new[:], in0=t1[:], in1=t2[:], op=OP.add)
                c_cur = c_new
                tc2 = sb.tile([B, HS], F32, tag="tc2")
                nc.scalar.activation(tc2[:], c_new[:], AF.Tanh)
                h2 = sb.tile([B, HS], F32, tag="h2")
                nc.vector.tensor_tensor(out=h2[:], in0=sig[:, 2 * HS:3 * HS], in1=tc2[:], op=OP.mult)

                # ---- h2 -> h2T (DVE), AllGather full h ----
                h2T = sb.tile([HS, B], F32, tag="h2T")
                _dve_transpose(nc, h2T[:], h2[:], B, HS)
                agi = dramp.tile([HS, B], F32, tag="agi")
                nc.sync.dma_start(agi[:], h2T[:])
                ago = dramp.tile([H, B], F32, tag="ago", addr_space="Shared")
                nc.gpsimd.collective_compute(
                    "AllGather", OP.bypass, replica_groups=rg,
                    ins=[agi[:].opt()], outs=[ago[:].opt()])
                hT_sb = sb.tile([128, 8, B], F32, tag="hT")
                nc.sync.dma_start(hT_sb[:], ago[:].rearrange("(k p) b -> p k b", p=128))

                # ---- split h into bf16 hi/lo (hi + 2^-11 * lo ~ 2^-20 accurate) ----
                hhi = sb.tile([128, 8, B], BF16, tag="hhi")
                nc.vector.tensor_copy(out=hhi[:], in_=hT_sb[:])
                hdf = sb.tile([128, 8, B], F32, tag="hdf")
                nc.vector.tensor_tensor(out=hdf[:], in0=hT_sb[:], in1=hhi[:], op=OP.subtract)
                hlo = sb.tile([128, 8, B], BF16, tag="hlo")
                nc.vector.tensor_scalar(out=hlo[:], in0=hdf[:], scalar1=2048.0,
                                        scalar2=None, op0=OP.mult)

                # ---- output projection (vocab shard): bf16 3-pass split ----
                # logits = bias + hhi@whi + 2^-11 * (hhi@wlo + hlo@whi)
                logits_sb = lgp.tile([B, VS], F32, tag="lg")
                mx8_all = sb.tile([B, NCHUNK, 8], F32, tag="mx8a")
                ix8_all = sb.tile([B, NCHUNK, 8], U32, tag="ix8a")
                for nci in range(NCHUNK):
                    cs = slice(nci * CW, (nci + 1) * CW)
                    psA = psl.tile([B, CW], F32, tag="psA")
                    nc.tensor.matmul(psA[:], lhsT=ones3[:], rhs=bo3_sb[:, cs],
                                     start=True, stop=False)
                    for k in range(8):
                        nc.tensor.matmul(psA[:], lhsT=hhi[:, k, :],
                                         rhs=whi_sb[:, k, cs],
                                         start=False, stop=(k == 7))
                    psB = psl.tile([B, CW], F32, tag="psB")
                    for k in range(8):
                        nc.tensor.matmul(psB[:], lhsT=hhi[:, k, :],
                                         rhs=wlo_sb[:, k, cs],
                                         start=(k == 0), stop=False)
                    for k in range(8):
                        nc.tensor.matmul(psB[:], lhsT=hlo[:, k, :],
                                         rhs=whi_sb[:, k, cs],
                                         start=False, stop=(k == 7))
                    tmp = sb.tile([B, CW], F32, tag="tmpc")
                    nc.vector.tensor_scalar(out=tmp[:], in0=psB[:], scalar1=1.0 / 2048.0,
                                            scalar2=None, op0=OP.mult)
                    nc.vector.tensor_tensor(out=logits_sb[:, cs], in0=tmp[:],
                                            in1=psA[:], op=OP.add)
                    nc.vector.max(out=mx8_all[:, nci, :], in_=logits_sb[:, cs])
                    nc.vector.max_index(out=ix8_all[:, nci, :],
                                        in_max=mx8_all[:, nci, :],
                                        in_values=logits_sb[:, cs])
                nc.sync.dma_start(o_decs[:, s, :], logits_sb[:])

                # ---- merge chunk argmaxes (local shard winner) ----
                vals = mx8_all[:, :, 0]                       # [B, 8] stride-8 AP
                ix8f = sb.tile([B, 8], F32, tag="ix8f")
                nc.vector.tensor_copy(out=ix8f[:], in_=ix8_all[:, :, 0])
                idxg = sb.tile([B, 8], F32, tag="idxg")
                nc.vector.tensor_tensor(out=idxg[:], in0=ix8f[:], in1=choff[:], op=OP.add)
                gm8 = sb.tile([B, 8], F32, tag="gm8")
                nc.vector.max(out=gm8[:], in_=vals)
                eq = sb.tile([B, 8], U32, tag="eq")
                nc.vector.tensor_scalar(out=eq[:], in0=vals, scalar1=gm8[:, 0:1],
                                        scalar2=None, op0=OP.is_equal)
                sel = sb.tile([B, 8], F32, tag="sel")
                nc.vector.select(out=sel[:], mask=eq[:], on_true=idxg[:], on_false=big[:])
                neg = sb.tile([B, 8], F32, tag="neg")
                nc.vector.tensor_scalar(out=neg[:], in0=sel[:], scalar1=-1.0,
                                        scalar2=None, op0=OP.mult)
                nm8 = sb.tile([B, 8], F32, tag="nm8")
                nc.vector.max(out=nm8[:], in_=neg[:])
                negl = sb.tile([B, 1], F32, tag="negl")
                nc.vector.tensor_scalar(out=negl[:], in0=nm8[:, 0:1], scalar1=-1.0,
                                        scalar2=None, op0=OP.mult)
                gidx = sb.tile([B, 1], F32, tag="gidx")
                nc.vector.tensor_scalar(out=gidx[:], in0=negl[:], scalar1=voff_sb[:, 0:1],
                                        scalar2=None, op0=OP.add)

                # ---- exchange (value, index) pairs, pick global argmax ----
                pair = sb.tile([B, 32], F32, tag="pair")
                nc.vector.tensor_copy(out=pair[:, 0:1], in_=gm8[:, 0:1])
                nc.vector.tensor_copy(out=pair[:, 1:2], in_=gidx[:])
                pairT = sb.tile([32, B], F32, tag="pairT")
                nc.vector.transpose(out=pairT[:, 0:32], in_=pair[0:32, :])
                nc.vector.transpose(out=pairT[:, 32:64], in_=pair[32:64, :])
                agi2 = dramp.tile([2, B], F32, tag="agi2")
                nc.sync.dma_start(agi2[:], pairT[0:2, :])
                ago2 = dramp.tile([2 * NC, B], F32, tag="ago2", addr_space="Shared")
                nc.gpsimd.collective_compute(
                    "AllGather", OP.bypass, replica_groups=rg,
                    ins=[agi2[:].opt()], outs=[ago2[:].opt()])
                cand32 = sb.tile([32, B], F32, tag="cand")
                nc.sync.dma_start(cand32[0:2 * NC, :], ago2[:])
                candT = sb.tile([B, 32], F32, tag="candT")
                nc.vector.transpose(out=candT[0:32, :], in_=cand32[:, 0:32])
                nc.vector.transpose(out=candT[32:64, :], in_=cand32[:, 32:64])

                cv = candT[:, 0:2 * NC].rearrange("b (c two) -> b c two", two=2)
                vals8 = cv[:, :, 0]
                idxs8 = cv[:, :, 1]
                gm8b = sb.tile([B, 8], F32, tag="gm8b")
                nc.vector.max(out=gm8b[:], in_=vals8)
                eqb = sb.tile([B, 8], U32, tag="eqb")
                nc.vector.tensor_scalar(out=eqb[:], in0=vals8, scalar1=gm8b[:, 0:1],
                                        scalar2=None, op0=OP.is_equal)
                selb = sb.tile([B, 8], F32, tag="selb")
                nc.vector.select(out=selb[:], mask=eqb[:], on_true=idxs8, on_false=big[:])
                negb = sb.tile([B, 8], F32, tag="negb")
                nc.vector.tensor_scalar(out=negb[:], in0=selb[:], scalar1=-1.0,
                                        scalar2=None, op0=OP.mult)
                nmb = sb.tile([B, 8], F32, tag="nmb")
                nc.vector.max(out=nmb[:], in_=negb[:])
                symf = sb.tile([B, 1], F32, tag="symf")
                nc.vector.tensor_scalar(out=symf[:], in0=nmb[:, 0:1], scalar1=-1.0,
                                        scalar2=None, op0=OP.mult)
                nc.vector.tensor_copy(out=syms_acc[:, s:s + 1], in_=symf[:])
                if s < S - 1:
                    sym_u32 = sb.tile([B, 1], U32, tag="symu")
                    nc.vector.tensor_copy(out=sym_u32[:], in_=symf[:])

            nc.sync.dma_start(o_syms, syms_acc[:])

    nc.compile()
    return nc


def _get_program(S):
    if S not in _cache:
        _cache[S] = _build(S)
    return _cache[S]


def _install_ntff_hook():
    import sys, types
    if "antenv.axon_hooks" in sys.modules:
        return
    try:
        from trn_agent_boot.trn_boot import _ntff_profile_via_ctypes
        hook = _ntff_profile_via_ctypes("/opt/axon/libaxon_pjrt.so")
    except Exception:
        return
    import antenv
    mod = types.ModuleType("antenv.axon_hooks")
    mod._hook = hook
    mod.set_axon_ntff_profile_hook = lambda h: setattr(mod, "_hook", h)
    mod.get_axon_ntff_profile_hook = lambda: mod._hook
    sys.modules["antenv.axon_hooks"] = mod
    antenv.axon_hooks = mod


def kernel(output, h0, c0, target_outputs, target_lengths,
           embed, w_ih, w_hh, b_ih, b_hh, w_out, b_out):
    global LAST_EXEC_NS, LAST_RESULTS
    import concourse.bass_utils as bass_utils

    embed = np.asarray(embed, dtype=np.float32)
    w_ih = np.asarray(w_ih, dtype=np.float32)
    w_hh = np.asarray(w_hh, dtype=np.float32)
    b_ih = np.asarray(b_ih, dtype=np.float32)
    b_hh = np.asarray(b_hh, dtype=np.float32)
    w_out = np.asarray(w_out, dtype=np.float32)
    b_out = np.asarray(b_out, dtype=np.float32)
    h0 = np.asarray(h0, dtype=np.float32)
    c0 = np.asarray(c0, dtype=np.float32)
    tl = np.asarray(target_lengths)
    sym_dtype = np.asarray(target_outputs).dtype

    T = int(tl.max()) if tl.size else 0
    S = max(T - 1, 0)

    dec0 = (embed[SOS] @ w_out.T + b_out).astype(np.float32)      # [V]
    if T == 0:
        return np.zeros((B, 0, V), np.float32), np.zeros((B, 0), sym_dtype)

    decs = np.empty((B, T, V), np.float32)
    decs[:, 0, :] = dec0[None, :]
    syms = np.empty((B, T), np.int64)
    syms[:, 0] = SOS
    if S == 0:
        return decs, syms.astype(sym_dtype)

    Epp = (embed @ w_ih.T + (b_ih + b_hh)[None, :]).astype(np.float32)  # [V, 4H]
    h0T = np.ascontiguousarray(h0[0].T)                                  # [H, B]

    import ml_dtypes
    bf16 = ml_dtypes.bfloat16

    in_maps = []
    for k in range(NC):
        perm = _gate_perm(k)
        woT_k = np.ascontiguousarray(w_out[k * VS:(k + 1) * VS, :].T)  # [H, VS] f32
        whi_k = woT_k.astype(bf16)
        wlo_k = ((woT_k - whi_k.astype(np.float32)) * 2048.0).astype(bf16)
        bo_k = b_out[k * VS:(k + 1) * VS].astype(np.float32)
        b1 = bo_k.astype(bf16)
        b2 = (bo_k - b1.astype(np.float32)).astype(bf16)
        b3 = (bo_k - b1.astype(np.float32) - b2.astype(np.float32)).astype(bf16)
        in_maps.append({
            "epp": np.ascontiguousarray(Epp[:, perm]),
            "wgT": np.ascontiguousarray(w_hh[perm, :].T),
            "whi": whi_k,
            "wlo": wlo_k,
            "bo3": np.stack([b1, b2, b3], axis=0),
            "h0T": h0T,
            "c0s": np.ascontiguousarray(c0[0][:, k * HS:(k + 1) * HS]),
            "voff": np.full((B, 1), k * VS, np.float32),
        })

    nc = _get_program(S)
    if TRACE:
        _install_ntff_hook()
    res = bass_utils.run_bass_kernel_spmd(
        nc, in_maps, core_ids=list(range(NC)), trace=TRACE)
    LAST_EXEC_NS = res.exec_time_ns
    LAST_RESULTS = res

    for k in range(NC):
        decs[:, 1:, k * VS:(k + 1) * VS] = res.results[k]["decs"]
    syms[:, 1:] = np.rint(res.results[0]["syms"]).astype(np.int64)
    return decs, syms.astype(sym_dtype)


# revision 11
# speedup vs baseline: 1.2540x; 1.0133x over previous
"""Greedy LSTM decoder (B=64, H=1024, V=32000, T<=40) on 8 Trainium2 cores.

Strategy (tensor-parallel over both H and V):
  - LSTM hidden dim sharded 8 ways: core k computes gates/h2/c2 for hidden
    units [k*128, (k+1)*128). The x-projection (embed[sym] @ w_ih.T + biases)
    is precomputed on the host as a table E'' = embed @ w_ih.T + b_ih + b_hh,
    so the per-step x-contribution is a 64-row indirect-DMA gather.
  - h is kept/AllGathered as a bf16 hi/lo split pair (hi + lo reconstructs h
    to ~2^-20 relative), and all matmuls run as 3-pass bf16 split products
    (hi@Whi + hi@Wlo + lo@Whi, fp32 PSUM accumulation). This matches the
    precision class of the PE's native fp32 (LOW_HIGH) mode at ~2.4x the
    throughput (1 cycle/row bf16 vs 4 cycles/row fp32).
  - Output projection sharded over vocab: core k holds w_out rows
    [k*4000, (k+1)*4000) (bf16 hi/lo, SBUF-resident) and computes its logit
    shard [64, 4000] per step, plus a local argmax (DVE max/max_index,
    chunked so it hides under the projection matmuls).
  - Local (value, index) argmax pairs are AllGathered and every core picks the
    global greedy symbol, which indexes the next gather.

Precision note: the reference trajectory has top-2 logit gaps down to 4e-7;
any argmax flip diverges the sequence, so every matmul feeding the argmax
keeps >=2^-20-class accuracy (verified: 0/2560 symbol flips vs fp32 jax).
"""

import numpy as np

SOS = 1
B, H, V = 64, 1024, 32000
NC = 8
VS = V // NC          # 4000 vocab shard
HS = H // NC          # 128 hidden shard
GS = 4 * HS           # 512 gate slice per core
NCHUNK = 8
CW = VS // NCHUNK     # 500 logit chunk width

_cache = {}           # S -> compiled program
TRACE = False         # test harness may set kernel.TRACE = True
LAST_EXEC_NS = None
LAST_RESULTS = None


def _gate_perm(k):
    """Rows of [4H, H]-shaped gate weight matrices owned by core k, reordered
    [i, f, o, g] so one sigmoid covers [0:384] and tanh covers [384:512]."""
    s = np.arange(k * HS, (k + 1) * HS)
    return np.concatenate([0 * H + s, 1 * H + s, 3 * H + s, 2 * H + s])


def _dve_transpose(nc, out, in_, p, f):
    """Global transpose in_[p, f] -> out[f, p] via DVE 32x32 block transposes."""
    for i in range(p // 32):
        for j in range(f // 32):
            nc.vector.transpose(out=out[j * 32:(j + 1) * 32, i * 32:(i + 1) * 32],
                                in_=in_[i * 32:(i + 1) * 32, j * 32:(j + 1) * 32])


def _build(S):
    """Build the bass program for S LSTM steps (t = 1..S)."""
    import concourse.bass as bass
    import concourse.bacc as bacc
    import concourse.tile as tile
    import concourse.mybir as mybir

    F32 = mybir.dt.float32
    BF16 = mybir.dt.bfloat16
    U32 = mybir.dt.uint32
    AF = mybir.ActivationFunctionType
    OP = mybir.AluOpType

    nc = bacc.Bacc("TRN2", target_bir_lowering=False, debug=False, num_devices=NC)

    epp = nc.dram_tensor("epp", [V, GS], F32, kind="ExternalInput").ap()
    wghi = nc.dram_tensor("wghi", [H, GS], BF16, kind="ExternalInput").ap()
    wglo = nc.dram_tensor("wglo", [H, GS], BF16, kind="ExternalInput").ap()
    whi = nc.dram_tensor("whi", [H, VS], BF16, kind="ExternalInput").ap()
    wlo = nc.dram_tensor("wlo", [H, VS], BF16, kind="ExternalInput").ap()
    bo3 = nc.dram_tensor("bo3", [3, VS], BF16, kind="ExternalInput").ap()
    h0T = nc.dram_tensor("h0T", [H, B], F32, kind="ExternalInput").ap()
    c0s = nc.dram_tensor("c0s", [B, HS], F32, kind="ExternalInput").ap()
    voff = nc.dram_tensor("voff", [B, 1], F32, kind="ExternalInput").ap()
    o_decs = nc.dram_tensor("decs", [B, S, VS], F32, kind="ExternalOutput").ap()
    o_syms = nc.dram_tensor("syms", [B, S], F32, kind="ExternalOutput").ap()

    rg = [list(range(NC))]

    with tile.TileContext(nc) as tc:
        with (
            tc.tile_pool(name="cst", bufs=1) as cst,
            tc.tile_pool(name="sb", bufs=2) as sb,
            tc.tile_pool(name="lgp", bufs=1) as lgp,
            tc.tile_pool(name="psg", bufs=2, space="PSUM") as psg,
            tc.tile_pool(name="psl", bufs=3, space="PSUM") as psl,
            tc.tile_pool(name="dram", bufs=3, space="DRAM") as dramp,
        ):
            # ---- persistent/resident data ----
            whi_sb = cst.tile([128, 8, VS], BF16)
            nc.sync.dma_start(whi_sb[:], whi.rearrange("(k p) n -> p k n", p=128))
            wlo_sb = cst.tile([128, 8, VS], BF16)
            nc.sync.dma_start(wlo_sb[:], wlo.rearrange("(k p) n -> p k n", p=128))
            wghi_sb = cst.tile([128, 8, GS], BF16)
            nc.sync.dma_start(wghi_sb[:], wghi.rearrange("(k p) n -> p k n", p=128))
            wglo_sb = cst.tile([128, 8, GS], BF16)
            nc.sync.dma_start(wglo_sb[:], wglo.rearrange("(k p) n -> p k n", p=128))
            bo3_sb = cst.tile([3, VS], BF16)
            nc.sync.dma_start(bo3_sb[:], bo3)
            ones3 = cst.tile([3, B], BF16)
            nc.vector.memset(ones3[:], 1.0)
            voff_sb = cst.tile([B, 1], F32)
            nc.sync.dma_start(voff_sb[:], voff)
            big = cst.tile([B, 8], F32)
            nc.vector.memset(big[:], 1.0e9)
            choff = cst.tile([B, 8], F32)
            for c in range(NCHUNK):
                nc.vector.memset(choff[:, c:c + 1], float(c * CW))
            syms_acc = cst.tile([B, S], F32)

            # initial h (bf16 split) from h0T
            h0_sb = sb.tile([128, 8, B], F32, tag="h0f")
            nc.sync.dma_start(h0_sb[:], h0T.rearrange("(k p) b -> p k b", p=128))
            hhi = sb.tile([128, 8, B], BF16, tag="hhi")
            nc.vector.tensor_copy(out=hhi[:], in_=h0_sb[:])
            hlo = sb.tile([128, 8, B], BF16, tag="hlo")
            nc.vector.tensor_tensor(out=hlo[:], in0=h0_sb[:], in1=hhi[:], op=OP.subtract)

            c_cur = sb.tile([B, HS], F32, tag="c")
            nc.sync.dma_start(c_cur[:], c0s)
            sym_u32 = sb.tile([B, 1], U32, tag="symu")
            nc.vector.memset(sym_u32[:], SOS)

            for s in range(S):
                # ---- gates-x gather: gx = E''[sym]  [B, GS] ----
                gx = sb.tile([B, GS], F32, tag="gx")
                nc.gpsimd.indirect_dma_start(
                    out=gx[:], out_offset=None, in_=epp,
                    in_offset=bass.IndirectOffsetOnAxis(ap=sym_u32[:, :1], axis=0))

                # ---- gates-h matmul (bf16 3-pass split) ----
                g_ps = psg.tile([B, GS], F32, tag="g")
                for k in range(8):
                    nc.tensor.matmul(g_ps[:], lhsT=hhi[:, k, :], rhs=wghi_sb[:, k, :],
                                     start=(k == 0), stop=False)
                for k in range(8):
                    nc.tensor.matmul(g_ps[:], lhsT=hhi[:, k, :], rhs=wglo_sb[:, k, :],
                                     start=False, stop=False)
                for k in range(8):
                    nc.tensor.matmul(g_ps[:], lhsT=hlo[:, k, :], rhs=wghi_sb[:, k, :],
                                     start=False, stop=(k == 7))
                g_sb = sb.tile([B, GS], F32, tag="gsb")
                nc.vector.tensor_tensor(out=g_sb[:], in0=g_ps[:], in1=gx[:], op=OP.add)

                # ---- LSTM elementwise (gate order i,f,o,g) ----
                sig = sb.tile([B, 3 * HS], F32, tag="sig")
                nc.scalar.activation(sig[:], g_sb[:, 0:3 * HS], AF.Sigmoid)
                tg = sb.tile([B, HS], F32, tag="tg")
                nc.scalar.activation(tg[:], g_sb[:, 3 * HS:4 * HS], AF.Tanh)
                t1 = sb.tile([B, HS], F32, tag="t1")
                nc.vector.tensor_tensor(out=t1[:], in0=sig[:, 0:HS], in1=tg[:], op=OP.mult)
                t2 = sb.tile([B, HS], F32, tag="t2")
                nc.vector.tensor_tensor(out=t2[:], in0=sig[:, HS:2 * HS], in1=c_cur[:], op=OP.mult)
                c_new = sb.tile([B, HS], F32, tag="c")
                nc.vector.tensor_tensor(out=c_new[:], in0=t1[:], in1=t2[:], op=OP.add)
                c_cur = c_new
                tc2 = sb.tile([B, HS], F32, tag="tc2")
                nc.scalar.activation(tc2[:], c_new[:], AF.Tanh)
                h2 = sb.tile([B, HS], F32, tag="h2")
                nc.vector.tensor_tensor(out=h2[:], in0=sig[:, 2 * HS:3 * HS], in1=tc2[:], op=OP.mult)

                # ---- h2 -> h2T (DVE), split to bf16 hi/lo, AllGather both ----
                h2T = sb.tile([HS, B], F32, tag="h2T")
                _dve_transpose(nc, h2T[:], h2[:], B, HS)
                h2hi = sb.tile([HS, B], BF16, tag="h2hi")
                nc.vector.tensor_copy(out=h2hi[:], in_=h2T[:])
                h2lo = sb.tile([HS, B], BF16, tag="h2lo")
                nc.vector.tensor_tensor(out=h2lo[:], in0=h2T[:], in1=h2hi[:], op=OP.subtract)
                agi = dramp.tile([HS, 2 * B], BF16, tag="agi")
                nc.sync.dma_start(agi[:, 0:B], h2hi[:])
                nc.sync.dma_start(agi[:, B:2 * B], h2lo[:])
                ago = dramp.tile([H, 2 * B], BF16, tag="ago", addr_space="Shared")
                nc.gpsimd.collective_compute(
                    "AllGather", OP.bypass, replica_groups=rg,
                    ins=[agi[:].opt()], outs=[ago[:].opt()])
                hhi = sb.tile([128, 8, B], BF16, tag="hhi")
                nc.sync.dma_start(
                    hhi[:], ago[:].rearrange("(k p) c -> p k c", p=128)[:, :, 0:B])
                hlo = sb.tile([128, 8, B], BF16, tag="hlo")
                nc.sync.dma_start(
                    hlo[:], ago[:].rearrange("(k p) c -> p k c", p=128)[:, :, B:2 * B])

                # ---- output projection (vocab shard): bf16 3-pass split ----
                logits_sb = lgp.tile([B, VS], F32, tag="lg")
                mx8_all = sb.tile([B, NCHUNK, 8], F32, tag="mx8a")
                ix8_all = sb.tile([B, NCHUNK, 8], U32, tag="ix8a")
                for nci in range(NCHUNK):
                    cs = slice(nci * CW, (nci + 1) * CW)
                    psA = psl.tile([B, CW], F32, tag="psA")
                    nc.tensor.matmul(psA[:], lhsT=ones3[:], rhs=bo3_sb[:, cs],
                                     start=True, stop=False)
                    for k in range(8):
                        nc.tensor.matmul(psA[:], lhsT=hhi[:, k, :],
                                         rhs=whi_sb[:, k, cs], start=False, stop=False)
                    for k in range(8):
                        nc.tensor.matmul(psA[:], lhsT=hhi[:, k, :],
                                         rhs=wlo_sb[:, k, cs], start=False, stop=False)
                    for k in range(8):
                        nc.tensor.matmul(psA[:], lhsT=hlo[:, k, :],
                                         rhs=whi_sb[:, k, cs], start=False, stop=(k == 7))
                    nc.vector.tensor_copy(out=logits_sb[:, cs], in_=psA[:])
                    nc.vector.max(out=mx8_all[:, nci, :], in_=logits_sb[:, cs])
                    nc.vector.max_index(out=ix8_all[:, nci, :],
                                        in_max=mx8_all[:, nci, :],
                                        in_values=logits_sb[:, cs])
                nc.sync.dma_start(o_decs[:, s, :], logits_sb[:])

                # ---- merge chunk argmaxes (local shard winner) ----
                vals = mx8_all[:, :, 0]                       # [B, 8] stride-8 AP
                ix8f = sb.tile([B, 8], F32, tag="ix8f")
                nc.vector.tensor_copy(out=ix8f[:], in_=ix8_all[:, :, 0])
                idxg = sb.tile([B, 8], F32, tag="idxg")
                nc.vector.tensor_tensor(out=idxg[:], in0=ix8f[:], in1=choff[:], op=OP.add)
                gm8 = sb.tile([B, 8], F32, tag="gm8")
                nc.vector.max(out=gm8[:], in_=vals)
                eq = sb.tile([B, 8], U32, tag="eq")
                nc.vector.tensor_scalar(out=eq[:], in0=vals, scalar1=gm8[:, 0:1],
                                        scalar2=None, op0=OP.is_equal)
                sel = sb.tile([B, 8], F32, tag="sel")
                nc.vector.select(out=sel[:], mask=eq[:], on_true=idxg[:], on_false=big[:])
                neg = sb.tile([B, 8], F32, tag="neg")
                nc.vector.tensor_scalar(out=neg[:], in0=sel[:], scalar1=-1.0,
                                        scalar2=None, op0=OP.mult)
                nm8 = sb.tile([B, 8], F32, tag="nm8")
                nc.vector.max(out=nm8[:], in_=neg[:])
                negl = sb.tile([B, 1], F32, tag="negl")
                nc.vector.tensor_scalar(out=negl[:], in0=nm8[:, 0:1], scalar1=-1.0,
                                        scalar2=None, op0=OP.mult)
                gidx = sb.tile([B, 1], F32, tag="gidx")
                nc.vector.tensor_scalar(out=gidx[:], in0=negl[:], scalar1=voff_sb[:, 0:1],
                                        scalar2=None, op0=OP.add)

                # ---- exchange (value, index) pairs, pick global argmax ----
                pair = sb.tile([B, 32], F32, tag="pair")
                nc.vector.tensor_copy(out=pair[:, 0:1], in_=gm8[:, 0:1])
                nc.vector.tensor_copy(out=pair[:, 1:2], in_=gidx[:])
                pairT = sb.tile([32, B], F32, tag="pairT")
                nc.vector.transpose(out=pairT[:, 0:32], in_=pair[0:32, :])
                nc.vector.transpose(out=pairT[:, 32:64], in_=pair[32:64, :])
                agi2 = dramp.tile([2, B], F32, tag="agi2")
                nc.sync.dma_start(agi2[:], pairT[0:2, :])
                ago2 = dramp.tile([2 * NC, B], F32, tag="ago2", addr_space="Shared")
                nc.gpsimd.collective_compute(
                    "AllGather", OP.bypass, replica_groups=rg,
                    ins=[agi2[:].opt()], outs=[ago2[:].opt()])
                cand32 = sb.tile([32, B], F32, tag="cand")
                nc.sync.dma_start(cand32[0:2 * NC, :], ago2[:])
                candT = sb.tile([B, 32], F32, tag="candT")
                nc.vector.transpose(out=candT[0:32, :], in_=cand32[:, 0:32])
                nc.vector.transpose(out=candT[32:64, :], in_=cand32[:, 32:64])

                cv = candT[:, 0:2 * NC].rearrange("b (c two) -> b c two", two=2)
                vals8 = cv[:, :, 0]
                idxs8 = cv[:, :, 1]
                gm8b = sb.tile([B, 8], F32, tag="gm8b")
                nc.vector.max(out=gm8b[:], in_=vals8)
                eqb = sb.tile([B, 8], U32, tag="eqb")
                nc.vector.tensor_scalar(out=eqb[:], in0=vals8, scalar1=gm8b[:, 0:1],
                                        scalar2=None, op0=OP.is_equal)
                selb = sb.tile([B, 8], F32, tag="selb")
                nc.vector.select(out=selb[:], mask=eqb[:], on_true=idxs8, on_false=big[:])
                negb = sb.tile([B, 8], F32, tag="negb")
                nc.vector.tensor_scalar(out=negb[:], in0=selb[:], scalar1=-1.0,
                                        scalar2=None, op0=OP.mult)
                nmb = sb.tile([B, 8], F32, tag="nmb")
                nc.vector.max(out=nmb[:], in_=negb[:])
                symf = sb.tile([B, 1], F32, tag="symf")
                nc.vector.tensor_scalar(out=symf[:], in0=nmb[:, 0:1], scalar1=-1.0,
                                        scalar2=None, op0=OP.mult)
                nc.vector.tensor_copy(out=syms_acc[:, s:s + 1], in_=symf[:])
                if s < S - 1:
                    sym_u32 = sb.tile([B, 1], U32, tag="symu")
                    nc.vector.tensor_copy(out=sym_u32[:], in_=symf[:])

            nc.sync.dma_start(o_syms, syms_acc[:])

    nc.compile()
    return nc


def _get_program(S):
    if S not in _cache:
        _cache[S] = _build(S)
    return _cache[S]


def _install_ntff_hook():
    import sys, types
    if "antenv.axon_hooks" in sys.modules:
        return
    try:
        from trn_agent_boot.trn_boot import _ntff_profile_via_ctypes
        hook = _ntff_profile_via_ctypes("/opt/axon/libaxon_pjrt.so")
    except Exception:
        return
    import antenv
    mod = types.ModuleType("antenv.axon_hooks")
    mod._hook = hook
    mod.set_axon_ntff_profile_hook = lambda h: setattr(mod, "_hook", h)
    mod.get_axon_ntff_profile_hook = lambda: mod._hook
    sys.modules["antenv.axon_hooks"] = mod
    antenv.axon_hooks = mod


def kernel(output, h0, c0, target_outputs, target_lengths,
           embed, w_ih, w_hh, b_ih, b_hh, w_out, b_out):
    global LAST_EXEC_NS, LAST_RESULTS
    import concourse.bass_utils as bass_utils

    embed = np.asarray(embed, dtype=np.float32)
    w_ih = np.asarray(w_ih, dtype=np.float32)
    w_hh = np.asarray(w_hh, dtype=np.float32)
    b_ih = np.asarray(b_ih, dtype=np.float32)
    b_hh = np.asarray(b_hh, dtype=np.float32)
    w_out = np.asarray(w_out, dtype=np.float32)
    b_out = np.asarray(b_out, dtype=np.float32)
    h0 = np.asarray(h0, dtype=np.float32)
    c0 = np.asarray(c0, dtype=np.float32)
    tl = np.asarray(target_lengths)
    sym_dtype = np.asarray(target_outputs).dtype

    T = int(tl.max()) if tl.size else 0
    S = max(T - 1, 0)

    dec0 = (embed[SOS] @ w_out.T + b_out).astype(np.float32)      # [V]
    if T == 0:
        return np.zeros((B, 0, V), np.float32), np.zeros((B, 0), sym_dtype)

    decs = np.empty((B, T, V), np.float32)
    decs[:, 0, :] = dec0[None, :]
    syms = np.empty((B, T), np.int64)
    syms[:, 0] = SOS
    if S == 0:
        return decs, syms.astype(sym_dtype)

    Epp = (embed @ w_ih.T + (b_ih + b_hh)[None, :]).astype(np.float32)  # [V, 4H]
    h0T = np.ascontiguousarray(h0[0].T)                                  # [H, B]

    import ml_dtypes
    bf16 = ml_dtypes.bfloat16

    def split(a):
        hi = a.astype(bf16)
        lo = (a - hi.astype(np.float32)).astype(bf16)
        return hi, lo

    in_maps = []
    for k in range(NC):
        perm = _gate_perm(k)
        whi_k, wlo_k = split(np.ascontiguousarray(w_out[k * VS:(k + 1) * VS, :].T))
        wghi_k, wglo_k = split(np.ascontiguousarray(w_hh[perm, :].T))
        bo_k = b_out[k * VS:(k + 1) * VS].astype(np.float32)
        b1 = bo_k.astype(bf16)
        b2 = (bo_k - b1.astype(np.float32)).astype(bf16)
        b3 = (bo_k - b1.astype(np.float32) - b2.astype(np.float32)).astype(bf16)
        in_maps.append({
            "epp": np.ascontiguousarray(Epp[:, perm]),
            "wghi": wghi_k,
            "wglo": wglo_k,
            "whi": whi_k,
            "wlo": wlo_k,
            "bo3": np.stack([b1, b2, b3], axis=0),
            "h0T": h0T,
            "c0s": np.ascontiguousarray(c0[0][:, k * HS:(k + 1) * HS]),
            "voff": np.full((B, 1), k * VS, np.float32),
        })

    nc = _get_program(S)
    if TRACE:
        _install_ntff_hook()
    res = bass_utils.run_bass_kernel_spmd(
        nc, in_maps, core_ids=list(range(NC)), trace=TRACE)
    LAST_EXEC_NS = res.exec_time_ns
    LAST_RESULTS = res

    for k in range(NC):
        decs[:, 1:, k * VS:(k + 1) * VS] = res.results[k]["decs"]
    syms[:, 1:] = np.rint(res.results[0]["syms"]).astype(np.int64)
    return decs, syms.astype(sym_dtype)


# revision 16
# speedup vs baseline: 18.6216x; 14.8500x over previous
"""Greedy LSTM decoder (B=64, H=1024, V=32000, T<=40) on 8 Trainium2 cores.

Strategy (tensor-parallel over both H and V):
  - LSTM hidden dim sharded 8 ways: core k computes gates/h2/c2 for hidden
    units [k*128, (k+1)*128). The x-projection (embed[sym] @ w_ih.T + biases)
    is precomputed on the host as a table E'' = embed @ w_ih.T + b_ih + b_hh,
    so the per-step x-contribution is a 64-row indirect-DMA gather.
  - h is kept/AllGathered as a bf16 hi/lo split pair (hi + lo reconstructs h
    to ~2^-20 relative), and all matmuls run as 3-pass bf16 split products
    (hi@Whi + hi@Wlo + lo@Whi, fp32 PSUM accumulation). This matches the
    precision class of the PE's native fp32 (LOW_HIGH) mode at ~2.4x the
    throughput (1 cycle/row bf16 vs 4 cycles/row fp32).
  - Output projection sharded over vocab: core k holds w_out rows
    [k*4000, (k+1)*4000) (bf16 hi/lo, SBUF-resident) and computes its logit
    shard [64, 4000] per step, plus a local argmax (DVE max/max_index,
    chunked so it hides under the projection matmuls).
  - Local (value, index) argmax pairs are AllGathered and every core picks the
    global greedy symbol, which indexes the next gather.

Precision note: the reference trajectory has top-2 logit gaps down to 4e-7;
any argmax flip diverges the sequence, so every matmul feeding the argmax
keeps >=2^-20-class accuracy (verified: 0/2560 symbol flips vs fp32 jax).
"""

import numpy as np

SOS = 1
B, H, V = 64, 1024, 32000
NC = 8
VS = V // NC          # 4000 vocab shard
HS = H // NC          # 128 hidden shard
GS = 4 * HS           # 512 gate slice per core
NCHUNK = 8
CW = VS // NCHUNK     # 500 logit chunk width

_cache = {}           # S -> compiled program
TRACE = False         # test harness may set kernel.TRACE = True
LAST_EXEC_NS = None
LAST_RESULTS = None


def _gate_perm(k):
    """Rows of [4H, H]-shaped gate weight matrices owned by core k, reordered
    [i, f, o, g] so one sigmoid covers [0:384] and tanh covers [384:512]."""
    s = np.arange(k * HS, (k + 1) * HS)
    return np.concatenate([0 * H + s, 1 * H + s, 3 * H + s, 2 * H + s])


def _dve_transpose(nc, out, in_, p, f):
    """Global transpose in_[p, f] -> out[f, p] via DVE 32x32 block transposes."""
    for i in range(p // 32):
        for j in range(f // 32):
            nc.vector.transpose(out=out[j * 32:(j + 1) * 32, i * 32:(i + 1) * 32],
                                in_=in_[i * 32:(i + 1) * 32, j * 32:(j + 1) * 32])


def _build(S):
    """Build the bass program for S LSTM steps (t = 1..S)."""
    import concourse.bass as bass
    import concourse.bacc as bacc
    import concourse.tile as tile
    import concourse.mybir as mybir
    from concourse.masks import make_identity

    F32 = mybir.dt.float32
    BF16 = mybir.dt.bfloat16
    U32 = mybir.dt.uint32
    AF = mybir.ActivationFunctionType
    OP = mybir.AluOpType

    nc = bacc.Bacc("TRN2", target_bir_lowering=False, debug=False, num_devices=NC)

    epp = nc.dram_tensor("epp", [V, GS], F32, kind="ExternalInput").ap()
    wghi = nc.dram_tensor("wghi", [H, GS], BF16, kind="ExternalInput").ap()
    wglo = nc.dram_tensor("wglo", [H, GS], BF16, kind="ExternalInput").ap()
    whi = nc.dram_tensor("whi", [H, VS], BF16, kind="ExternalInput").ap()
    wlo = nc.dram_tensor("wlo", [H, VS], BF16, kind="ExternalInput").ap()
    bo3 = nc.dram_tensor("bo3", [3, VS], BF16, kind="ExternalInput").ap()
    h0T = nc.dram_tensor("h0T", [H, B], F32, kind="ExternalInput").ap()
    c0s = nc.dram_tensor("c0s", [B, HS], F32, kind="ExternalInput").ap()
    voff = nc.dram_tensor("voff", [B, 1], F32, kind="ExternalInput").ap()
    o_decs = nc.dram_tensor("decs", [B, S, VS], F32, kind="ExternalOutput").ap()
    o_syms = nc.dram_tensor("syms", [B, S], F32, kind="ExternalOutput").ap()

    rg = [list(range(NC))]

    with tile.TileContext(nc) as tc:
        with (
            tc.tile_pool(name="cst", bufs=1) as cst,
            tc.tile_pool(name="sb", bufs=2) as sb,
            tc.tile_pool(name="lgp", bufs=1) as lgp,
            tc.tile_pool(name="psg", bufs=2, space="PSUM") as psg,
            tc.tile_pool(name="psl", bufs=3, space="PSUM") as psl,
            tc.tile_pool(name="pst", bufs=2, space="PSUM") as pst,
            tc.tile_pool(name="dram", bufs=3, space="DRAM") as dramp,
        ):
            # ---- persistent/resident data ----
            whi_sb = cst.tile([128, 8, VS], BF16)
            nc.sync.dma_start(whi_sb[:], whi.rearrange("(k p) n -> p k n", p=128))
            wlo_sb = cst.tile([128, 8, VS], BF16)
            nc.sync.dma_start(wlo_sb[:], wlo.rearrange("(k p) n -> p k n", p=128))
            wghi_sb = cst.tile([128, 8, GS], BF16)
            nc.sync.dma_start(wghi_sb[:], wghi.rearrange("(k p) n -> p k n", p=128))
            wglo_sb = cst.tile([128, 8, GS], BF16)
            nc.sync.dma_start(wglo_sb[:], wglo.rearrange("(k p) n -> p k n", p=128))
            bo3_sb = cst.tile([3, VS], BF16)
            nc.sync.dma_start(bo3_sb[:], bo3)
            ones3 = cst.tile([3, B], BF16)
            nc.vector.memset(ones3[:], 1.0)
            voff_sb = cst.tile([B, 1], F32)
            nc.sync.dma_start(voff_sb[:], voff)
            big = cst.tile([B, 8], F32)
            nc.vector.memset(big[:], 1.0e9)
            choff = cst.tile([B, 8], F32)
            for c in range(NCHUNK):
                nc.vector.memset(choff[:, c:c + 1], float(c * CW))
            ident = cst.tile([B, B], F32)
            make_identity(nc, ident[:])
            syms_acc = cst.tile([B, S], F32)

            # initial h (bf16 split) from h0T
            h0_sb = sb.tile([128, 8, B], F32, tag="h0f")
            nc.sync.dma_start(h0_sb[:], h0T.rearrange("(k p) b -> p k b", p=128))
            hhi = sb.tile([128, 8, B], BF16, tag="hhi")
            nc.vector.tensor_copy(out=hhi[:], in_=h0_sb[:])
            hlo = sb.tile([128, 8, B], BF16, tag="hlo")
            nc.vector.tensor_tensor(out=hlo[:], in0=h0_sb[:], in1=hhi[:], op=OP.subtract)

            c_cur = sb.tile([B, HS], F32, tag="c")
            nc.sync.dma_start(c_cur[:], c0s)
            sym_u32 = sb.tile([B, 1], U32, tag="symu")
            nc.vector.memset(sym_u32[:], SOS)

            for s in range(S):
                # ---- gates-x gather: gx = E''[sym]  [B, GS] ----
                gx = sb.tile([B, GS], F32, tag="gx")
                nc.gpsimd.indirect_dma_start(
                    out=gx[:], out_offset=None, in_=epp,
                    in_offset=bass.IndirectOffsetOnAxis(ap=sym_u32[:, :1], axis=0))

                # ---- gates-h matmul (bf16 3-pass split) ----
                g_ps = psg.tile([B, GS], F32, tag="g")
                for k in range(8):
                    nc.tensor.matmul(g_ps[:], lhsT=hhi[:, k, :], rhs=wghi_sb[:, k, :],
                                     start=(k == 0), stop=False)
                for k in range(8):
                    nc.tensor.matmul(g_ps[:], lhsT=hhi[:, k, :], rhs=wglo_sb[:, k, :],
                                     start=False, stop=False)
                for k in range(8):
                    nc.tensor.matmul(g_ps[:], lhsT=hlo[:, k, :], rhs=wghi_sb[:, k, :],
                                     start=False, stop=(k == 7))
                g_sb = sb.tile([B, GS], F32, tag="gsb")
                nc.vector.tensor_tensor(out=g_sb[:], in0=g_ps[:], in1=gx[:], op=OP.add)

                # ---- LSTM elementwise (gate order i,f,o,g) ----
                sig = sb.tile([B, 3 * HS], F32, tag="sig")
                nc.scalar.activation(sig[:], g_sb[:, 0:3 * HS], AF.Sigmoid)
                tg = sb.tile([B, HS], F32, tag="tg")
                nc.scalar.activation(tg[:], g_sb[:, 3 * HS:4 * HS], AF.Tanh)
                t1 = sb.tile([B, HS], F32, tag="t1")
                nc.vector.tensor_tensor(out=t1[:], in0=sig[:, 0:HS], in1=tg[:], op=OP.mult)
                t2 = sb.tile([B, HS], F32, tag="t2")
                nc.vector.tensor_tensor(out=t2[:], in0=sig[:, HS:2 * HS], in1=c_cur[:], op=OP.mult)
                c_new = sb.tile([B, HS], F32, tag="c")
                nc.vector.tensor_tensor(out=c_new[:], in0=t1[:], in1=t2[:], op=OP.add)
                c_cur = c_new
                tc2 = sb.tile([B, HS], F32, tag="tc2")
                nc.scalar.activation(tc2[:], c_new[:], AF.Tanh)
                h2 = sb.tile([B, HS], F32, tag="h2")
                nc.vector.tensor_tensor(out=h2[:], in0=sig[:, 2 * HS:3 * HS], in1=tc2[:], op=OP.mult)

                # ---- h2 -> h2T (PE), split to bf16 hi/lo, AllGather both ----
                trp = pst.tile([HS, B], F32, tag="trp")
                nc.tensor.transpose(out=trp[:], in_=h2[:], identity=ident[:])
                h2hi = sb.tile([HS, B], BF16, tag="h2hi")
                nc.vector.tensor_copy(out=h2hi[:], in_=trp[:])
                h2lo = sb.tile([HS, B], BF16, tag="h2lo")
                nc.vector.tensor_tensor(out=h2lo[:], in0=trp[:], in1=h2hi[:], op=OP.subtract)
                agi = dramp.tile([HS, 2 * B], BF16, tag="agi")
                nc.sync.dma_start(agi[:, 0:B], h2hi[:])
                nc.sync.dma_start(agi[:, B:2 * B], h2lo[:])
                ago = dramp.tile([H, 2 * B], BF16, tag="ago", addr_space="Shared")
                nc.gpsimd.collective_compute(
                    "AllGather", OP.bypass, replica_groups=rg,
                    ins=[agi[:].opt()], outs=[ago[:].opt()])
                hhi = sb.tile([128, 8, B], BF16, tag="hhi")
                nc.sync.dma_start(
                    hhi[:], ago[:].rearrange("(k p) c -> p k c", p=128)[:, :, 0:B])
                hlo = sb.tile([128, 8, B], BF16, tag="hlo")
                nc.sync.dma_start(
                    hlo[:], ago[:].rearrange("(k p) c -> p k c", p=128)[:, :, B:2 * B])

                # ---- output projection (vocab shard): bf16 3-pass split,
                #      two chunks col-tiled onto array halves concurrently ----
                logits_sb = lgp.tile([B, VS], F32, tag="lg")
                mx8_all = sb.tile([B, NCHUNK, 8], F32, tag="mx8a")
                ix8_all = sb.tile([B, NCHUNK, 8], U32, tag="ix8a")
                passes = [(hhi, whi_sb), (hhi, wlo_sb), (hlo, whi_sb)]
                for pi in range(NCHUNK // 2):
                    csA = slice((2 * pi) * CW, (2 * pi + 1) * CW)
                    csB = slice((2 * pi + 1) * CW, (2 * pi + 2) * CW)
                    psP = psl.tile([128, CW], F32, tag="psP")
                    nc.tensor.matmul(psP[0:B, :], lhsT=ones3[:], rhs=bo3_sb[:, csA],
                                     start=True, stop=False, tile_position=(0, 0),
                                     skip_group_check=True)
                    nc.tensor.matmul(psP[B:2 * B, :], lhsT=ones3[:], rhs=bo3_sb[:, csB],
                                     start=True, stop=False, tile_position=(0, 64),
                                     skip_group_check=True)
                    for lh, w_sb in passes:
                        last = (lh is hlo)
                        for k in range(8):
                            st = last and k == 7
                            nc.tensor.matmul(psP[0:B, :], lhsT=lh[:, k, :],
                                             rhs=w_sb[:, k, csA], start=False, stop=st,
                                             tile_position=(0, 0),
                                             skip_group_check=True)
                            nc.tensor.matmul(psP[B:2 * B, :], lhsT=lh[:, k, :],
                                             rhs=w_sb[:, k, csB], start=False, stop=st,
                                             tile_position=(0, 64),
                                             skip_group_check=True)
                    for half, cs, nci in ((0, csA, 2 * pi), (1, csB, 2 * pi + 1)):
                        ps = psP[half * B:(half + 1) * B, :]
                        nc.vector.tensor_copy(out=logits_sb[:, cs], in_=ps)
                        nc.vector.max(out=mx8_all[:, nci, :], in_=logits_sb[:, cs])
                        nc.vector.max_index(out=ix8_all[:, nci, :],
                                            in_max=mx8_all[:, nci, :],
                                            in_values=logits_sb[:, cs])
                nc.sync.dma_start(o_decs[:, s, :], logits_sb[:])

                # ---- merge chunk argmaxes (local shard winner) ----
                vals = mx8_all[:, :, 0]                       # [B, 8] stride-8 AP
                ix8f = sb.tile([B, 8], F32, tag="ix8f")
                nc.vector.tensor_copy(out=ix8f[:], in_=ix8_all[:, :, 0])
                idxg = sb.tile([B, 8], F32, tag="idxg")
                nc.vector.tensor_tensor(out=idxg[:], in0=ix8f[:], in1=choff[:], op=OP.add)
                gm8 = sb.tile([B, 8], F32, tag="gm8")
                nc.vector.max(out=gm8[:], in_=vals)
                eq = sb.tile([B, 8], U32, tag="eq")
                nc.vector.tensor_scalar(out=eq[:], in0=vals, scalar1=gm8[:, 0:1],
                                        scalar2=None, op0=OP.is_equal)
                sel = sb.tile([B, 8], F32, tag="sel")
                nc.vector.select(out=sel[:], mask=eq[:], on_true=idxg[:], on_false=big[:])
                neg = sb.tile([B, 8], F32, tag="neg")
                nc.vector.tensor_scalar(out=neg[:], in0=sel[:], scalar1=-1.0,
                                        scalar2=None, op0=OP.mult)
                nm8 = sb.tile([B, 8], F32, tag="nm8")
                nc.vector.max(out=nm8[:], in_=neg[:])
                negl = sb.tile([B, 1], F32, tag="negl")
                nc.vector.tensor_scalar(out=negl[:], in0=nm8[:, 0:1], scalar1=-1.0,
                                        scalar2=None, op0=OP.mult)
                gidx = sb.tile([B, 1], F32, tag="gidx")
                nc.vector.tensor_scalar(out=gidx[:], in0=negl[:], scalar1=voff_sb[:, 0:1],
                                        scalar2=None, op0=OP.add)

                # ---- exchange (value, index) pairs, pick global argmax ----
                pair = sb.tile([B, 32], F32, tag="pair")
                nc.vector.tensor_copy(out=pair[:, 0:1], in_=gm8[:, 0:1])
                nc.vector.tensor_copy(out=pair[:, 1:2], in_=gidx[:])
                pairT = sb.tile([32, B], F32, tag="pairT")
                nc.vector.transpose(out=pairT[:, 0:32], in_=pair[0:32, :])
                nc.vector.transpose(out=pairT[:, 32:64], in_=pair[32:64, :])
                agi2 = dramp.tile([2, B], F32, tag="agi2")
                nc.sync.dma_start(agi2[:], pairT[0:2, :])
                ago2 = dramp.tile([2 * NC, B], F32, tag="ago2", addr_space="Shared")
                nc.gpsimd.collective_compute(
                    "AllGather", OP.bypass, replica_groups=rg,
                    ins=[agi2[:].opt()], outs=[ago2[:].opt()])
                cand32 = sb.tile([32, B], F32, tag="cand")
                nc.sync.dma_start(cand32[0:2 * NC, :], ago2[:])
                candT = sb.tile([B, 32], F32, tag="candT")
                nc.vector.transpose(out=candT[0:32, :], in_=cand32[:, 0:32])
                nc.vector.transpose(out=candT[32:64, :], in_=cand32[:, 32:64])

                cv = candT[:, 0:2 * NC].rearrange("b (c two) -> b c two", two=2)
                vals8 = cv[:, :, 0]
                idxs8 = cv[:, :, 1]
                gm8b = sb.tile([B, 8], F32, tag="gm8b")
                nc.vector.max(out=gm8b[:], in_=vals8)
                eqb = sb.tile([B, 8], U32, tag="eqb")
                nc.vector.tensor_scalar(out=eqb[:], in0=vals8, scalar1=gm8b[:, 0:1],
                                        scalar2=None, op0=OP.is_equal)
                selb = sb.tile([B, 8], F32, tag="selb")
                nc.vector.select(out=selb[:], mask=eqb[:], on_true=idxs8, on_false=big[:])
                negb = sb.tile([B, 8], F32, tag="negb")
                nc.vector.tensor_scalar(out=negb[:], in0=selb[:], scalar1=-1.0,
                                        scalar2=None, op0=OP.mult)
                nmb = sb.tile([B, 8], F32, tag="nmb")
                nc.vector.max(out=nmb[:], in_=negb[:])
                symf = sb.tile([B, 1], F32, tag="symf")
                nc.vector.tensor_scalar(out=symf[:], in0=nmb[:, 0:1], scalar1=-1.0,
                                        scalar2=None, op0=OP.mult)
                nc.vector.tensor_copy(out=syms_acc[:, s:s + 1], in_=symf[:])
                if s < S - 1:
                    sym_u32 = sb.tile([B, 1], U32, tag="symu")
                    nc.vector.tensor_copy(out=sym_u32[:], in_=symf[:])

            nc.sync.dma_start(o_syms, syms_acc[:])

    nc.compile()
    return nc


def _get_program(S):
    if S not in _cache:
        _cache[S] = _build(S)
    return _cache[S]


def _install_ntff_hook():
    import sys, types
    if "antenv.axon_hooks" in sys.modules:
        return
    try:
        from trn_agent_boot.trn_boot import _ntff_profile_via_ctypes
        hook = _ntff_profile_via_ctypes("/opt/axon/libaxon_pjrt.so")
    except Exception:
        return
    import antenv
    mod = types.ModuleType("antenv.axon_hooks")
    mod._hook = hook
    mod.set_axon_ntff_profile_hook = lambda h: setattr(mod, "_hook", h)
    mod.get_axon_ntff_profile_hook = lambda: mod._hook
    sys.modules["antenv.axon_hooks"] = mod
    antenv.axon_hooks = mod


def kernel(output, h0, c0, target_outputs, target_lengths,
           embed, w_ih, w_hh, b_ih, b_hh, w_out, b_out):
    global LAST_EXEC_NS, LAST_RESULTS
    import concourse.bass_utils as bass_utils

    embed = np.asarray(embed, dtype=np.float32)
    w_ih = np.asarray(w_ih, dtype=np.float32)
    w_hh = np.asarray(w_hh, dtype=np.float32)
    b_ih = np.asarray(b_ih, dtype=np.float32)
    b_hh = np.asarray(b_hh, dtype=np.float32)
    w_out = np.asarray(w_out, dtype=np.float32)
    b_out = np.asarray(b_out, dtype=np.float32)
    h0 = np.asarray(h0, dtype=np.float32)
    c0 = np.asarray(c0, dtype=np.float32)
    tl = np.asarray(target_lengths)
    sym_dtype = np.asarray(target_outputs).dtype

    T = int(tl.max()) if tl.size else 0
    S = max(T - 1, 0)

    dec0 = (embed[SOS] @ w_out.T + b_out).astype(np.float32)      # [V]
    if T == 0:
        return np.zeros((B, 0, V), np.float32), np.zeros((B, 0), sym_dtype)

    decs = np.empty((B, T, V), np.float32)
    decs[:, 0, :] = dec0[None, :]
    syms = np.empty((B, T), np.int64)
    syms[:, 0] = SOS
    if S == 0:
        return decs, syms.astype(sym_dtype)

    Epp = (embed @ w_ih.T + (b_ih + b_hh)[None, :]).astype(np.float32)  # [V, 4H]
    h0T = np.ascontiguousarray(h0[0].T)                                  # [H, B]

    import ml_dtypes
    bf16 = ml_dtypes.bfloat16

    def split(a):
        hi = a.astype(bf16)
        lo = (a - hi.astype(np.float32)).astype(bf16)
        return hi, lo

    in_maps = []
    for k in range(NC):
        perm = _gate_perm(k)
        whi_k, wlo_k = split(np.ascontiguousarray(w_out[k * VS:(k + 1) * VS, :].T))
        wghi_k, wglo_k = split(np.ascontiguousarray(w_hh[perm, :].T))
        bo_k = b_out[k * VS:(k + 1) * VS].astype(np.float32)
        b1 = bo_k.astype(bf16)
        b2 = (bo_k - b1.astype(np.float32)).astype(bf16)
        b3 = (bo_k - b1.astype(np.float32) - b2.astype(np.float32)).astype(bf16)
        in_maps.append({
            "epp": np.ascontiguousarray(Epp[:, perm]),
            "wghi": wghi_k,
            "wglo": wglo_k,
            "whi": whi_k,
            "wlo": wlo_k,
            "bo3": np.stack([b1, b2, b3], axis=0),
            "h0T": h0T,
            "c0s": np.ascontiguousarray(c0[0][:, k * HS:(k + 1) * HS]),
            "voff": np.full((B, 1), k * VS, np.float32),
        })

    nc = _get_program(S)
    if TRACE:
        _install_ntff_hook()
    res = bass_utils.run_bass_kernel_spmd(
        nc, in_maps, core_ids=list(range(NC)), trace=TRACE)
    LAST_EXEC_NS = res.exec_time_ns
    LAST_RESULTS = res

    for k in range(NC):
        decs[:, 1:, k * VS:(k + 1) * VS] = res.results[k]["decs"]
    syms[:, 1:] = np.rint(res.results[0]["syms"]).astype(np.int64)
    return decs, syms.astype(sym_dtype)
